# revision 1
# baseline (speedup 1.0000x reference)
"""Distributed AstrometryConcordanceHead kernel for 8 Trainium2 NeuronCores.

Pure data parallel: batch B=8 sharded one sample per NeuronCore; tiny params
(Wr, Wv, log_temperature) replicated. Each core computes projection, local
cost volume (R=3 -> K=49 shifts) and soft-argmax independently.

Cost volume strategy: instead of 49 full-image multiply-reduce passes
(memory-bound: 49 re-reads of the projected tensors), compute per-dy banded
Gram matmuls over x-tiles (PE-friendly batched matmuls that read each
projected tensor only 7x) and extract the 7 in-band diagonals with a
constant 0/1 selection einsum (also a matmul - no gathers).
"""

import numpy as np

R = 3
K = (2 * R + 1) ** 2
EPS_NORM = 1e-6
TAU_MIN = 1e-3

B, D, H, W = 8, 256, 192, 192
Dm = 64

XT = 64               # x-tile width
NT = W // XT          # number of x-tiles
WZ = XT + 2 * R       # z-window per tile

_COMPILED = {}


def _build():
    import jax
    import jax.numpy as jnp

    devs = jax.devices()[:8]
    bf16 = jnp.bfloat16

    offs = np.arange(-R, R + 1, dtype=np.float32)
    dy_lut = np.repeat(offs, 2 * R + 1)   # [K]
    dx_lut = np.tile(offs, 2 * R + 1)     # [K]

    # SEL[x, z, d] = 1 where z == x + d  (z-window local coords; d = dx+3)
    sel = np.zeros((XT, WZ, 2 * R + 1), np.float32)
    for x in range(XT):
        for d in range(2 * R + 1):
            sel[x, x + d, d] = 1.0
    SEL = sel

    def per_core(rubin_2d, vis_2d, Wr, Wv, log_temperature):
        r = rubin_2d[0]   # [D, H, W]
        v = vis_2d[0]

        rub = jnp.einsum('dhw,md->mhw', r, Wr,
                         preferred_element_type=jnp.float32)
        vis = jnp.einsum('dhw,md->mhw', v, Wv,
                         preferred_element_type=jnp.float32)

        def l2n(x):
            n = jnp.sqrt(jnp.sum(x * x, axis=0, keepdims=True))
            return x / jnp.maximum(n, EPS_NORM)

        rub_n = l2n(rub)
        vis_n = l2n(vis)
        vis_pad = jnp.pad(vis_n, ((0, 0), (R, R), (R, R)), mode='edge')

        rub_b = rub_n.astype(bf16).reshape(Dm, H, NT, XT)        # m,y,t,x
        # per x-tile z-windows of the padded vis rows: [m, H+6, NT, WZ]
        vis_win = jnp.stack(
            [vis_pad[:, :, XT * t:XT * t + WZ] for t in range(NT)],
            axis=2).astype(bf16)

        selc = jnp.asarray(SEL, dtype=jnp.float32)
        scale = 1.0 / np.sqrt(float(Dm))
        tau = jnp.maximum(jnp.exp(log_temperature[0]), TAU_MIN)
        zscale = (scale / tau).astype(jnp.float32)

        exts = []
        for dy in range(2 * R + 1):
            vw = jax.lax.slice_in_dim(vis_win, dy, dy + H, axis=1)
            # banded Gram: batch (y,t), contract m -> [H, NT, XT, WZ]
            G = jnp.einsum('mytx,mytz->ytxz', rub_b, vw,
                           preferred_element_type=jnp.float32)
            # in-band diagonal extraction as matmul: [H, NT, XT, 7]
            ext = jnp.einsum('ytxz,xzd->ytxd', G, selc)
            exts.append(ext)
        # [H, NT, XT, 7dy, 7dx] -> [H, W, K]
        z = jnp.stack(exts, axis=3) * zscale
        z = z.reshape(H, W, K)

        zmax = jnp.max(z, axis=-1, keepdims=True)
        p = jnp.exp(z - zmax)
        s = jnp.sum(p, axis=-1, keepdims=True)
        inv_s = 1.0 / s[..., 0]                       # [H, W]
        conf_local = inv_s                            # max prob = 1/s
        dyl = jnp.asarray(dy_lut)
        dxl = jnp.asarray(dx_lut)
        dy_local = jnp.einsum('hwk,k->hw', p, dyl) * inv_s
        dx_local = jnp.einsum('hwk,k->hw', p, dxl) * inv_s

        # global softmax over the per-sample mean logits
        zg = jnp.mean(z, axis=(0, 1))                 # [K]
        zg = zg - jnp.max(zg)
        pg = jnp.exp(zg)
        sg = jnp.sum(pg)
        probs_g = pg / sg
        dy_g = jnp.sum(probs_g * dyl)
        dx_g = jnp.sum(probs_g * dxl)
        conf_g = jnp.max(probs_g)

        uniform = 1.0 / float(K)
        lw = jnp.clip((conf_local - uniform) / max(1e-6, 1.0 - uniform),
                      0.0, 1.0)
        dy_o = lw * dy_local + (1.0 - lw) * dy_g
        dx_o = lw * dx_local + (1.0 - lw) * dx_g
        conf_gb = jnp.broadcast_to(conf_g, (H, W))
        out = jnp.stack([dy_o, dx_o, conf_local, lw, conf_gb], axis=0)
        return out.astype(jnp.float32)                # [5, H, W]

    fn = jax.pmap(per_core, in_axes=(0, 0, None, None, None), devices=devs)
    return fn


def kernel(rubin_2d, vis_2d, Wr, Wv, log_temperature):
    if 'fn' not in _COMPILED:
        _COMPILED['fn'] = _build()
    fn = _COMPILED['fn']

    rubin = np.asarray(rubin_2d, dtype=np.float32).reshape(B, 1, D, H, W)
    vis = np.asarray(vis_2d, dtype=np.float32).reshape(B, 1, D, H, W)
    Wr = np.asarray(Wr, dtype=np.float32)
    Wv = np.asarray(Wv, dtype=np.float32)
    lt = np.asarray(log_temperature, dtype=np.float32)

    out = fn(rubin, vis, Wr, Wv, lt)   # [8, 5, H, W]
    return np.asarray(out).astype(np.float32)

